# revision 8
# baseline (speedup 1.0000x reference)
"""Gumbel-Sinkhorn (masked, 5 iterations) on Trainium2, data-parallel over 8 cores.

Math: the reference's masked log-domain Sinkhorn is equivalent, in the
probability domain, to classic Sinkhorn scaling of K = exp(masked_logits):

    v_0 = 1;   u_k = 1 / (K v_{k-1});   v_k = 1 / (K^T u_k)      (k = 1..5)
    out = K * (u_5 outer v_5) * exp(1e-6),  masked entries exactly 0.

Per core (64 samples), everything runs as PE matvecs with the sample matrix
as the stationary operand, batched DVE reciprocals across a cohort of
samples, and a final PE-broadcast + ACT/DVE materialization.

Host-side prep (cheap, O(B*A*T) numpy): mask logits to -1e4 (exp -> exact 0)
and build the per-sample transposed copy so both row- and column-phase
matvecs contract along the SBUF partition axis.
"""

import numpy as np

B, A, T = 512, 256, 256
NCORES = 8
BPC = B // NCORES          # samples per core
C = 16                     # cohort size (samples in lockstep)
ITERS = 5
MASKVAL = np.float32(-1e4)  # exp(-1e4) == 0.0 exactly in fp32
EPS = 1e-15                 # guards 1/0 on fully-masked rows/cols
OUT_SCALE = float(np.exp(np.float64(1e-6)))  # reference's exp(x + 1e-6)

_NC_CACHE = None


def _build_nc():
    import concourse.tile as tile
    from concourse import bacc, mybir

    f32 = mybir.dt.float32
    AF = mybir.ActivationFunctionType

    nc = bacc.Bacc()
    lg = nc.dram_tensor("lg", [BPC, A, T], f32, kind="ExternalInput")
    lgT = nc.dram_tensor("lgT", [BPC, T, A], f32, kind="ExternalInput")
    ident = nc.dram_tensor("ident", [128, 128], f32, kind="ExternalInput")
    # sel[k, b*128+m] = OUT_SCALE if k == b else 0: selector weights that turn
    # the [C,256] v-row tile into a per-sample [128,256] broadcast via PE.
    sel = nc.dram_tensor("sel", [C, C * 128], f32, kind="ExternalInput")
    out = nc.dram_tensor("out", [BPC, A, T], f32, kind="ExternalOutput")

    G = BPC // C  # number of cohorts
    SLAB = C * 512  # free elems per slab: per sample 2 halves x 256

    with tile.TileContext(nc) as tc:
        with (
            tc.tile_pool(name="e0p", bufs=2) as e0p,
            tc.tile_pool(name="etp", bufs=2) as etp,
            tc.tile_pool(name="uvp", bufs=10) as uvp,
            tc.tile_pool(name="wp", bufs=4) as wp,
            tc.tile_pool(name="vrowp", bufs=2) as vrowp,
            tc.tile_pool(name="constp", bufs=1) as constp,
            tc.tile_pool(name="psuv", bufs=2, space="PSUM") as psuv,
            tc.tile_pool(name="psbc", bufs=3, space="PSUM") as psbc,
            tc.tile_pool(name="psvr", bufs=1, space="PSUM") as psvr,
        ):
            ident_sb = constp.tile([128, 128], f32)
            nc.sync.dma_start(ident_sb[:], ident[:])
            sel_sb = constp.tile([C, C * 128], f32)
            nc.sync.dma_start(sel_sb[0:C, :], sel[:])
            v_ones = constp.tile([128, 2 * C], f32)
            nc.vector.memset(v_ones[:], 1.0)

            for g in range(G):
                s0 = g * C
                # ---- load + exp (masked logits -> K, K^T) ----
                e0 = e0p.tile([128, SLAB], f32, name="e0")
                et = etp.tile([128, SLAB], f32, name="et")
                src = lg[s0:s0 + C].rearrange("b (h p) j -> p b h j", p=128)
                nc.sync.dma_start(e0[:].rearrange("p (b h j) -> p b h j", h=2, j=256), src)
                srcT = lgT[s0:s0 + C].rearrange("b (h p) j -> p b h j", p=128)
                nc.sync.dma_start(et[:].rearrange("p (b h j) -> p b h j", h=2, j=256), srcT)
                nc.scalar.activation(e0[:], e0[:], AF.Exp)
                nc.scalar.activation(et[:], et[:], AF.Exp)

                # ---- 5 Sinkhorn iterations (scale vectors only) ----
                # layout of u/v tiles: [128, 2C], column h*C + b = half h of sample b
                v_cur = v_ones
                u_cur = None
                for it in range(ITERS):
                    ps_u = psuv.tile([128, 2 * C], f32, name="ps_u")
                    for b in range(C):
                        for ia in range(2):
                            for jt in range(2):
                                nc.tensor.matmul(
                                    ps_u[:, ia * C + b: ia * C + b + 1],
                                    lhsT=et[:, b * 512 + jt * 256 + ia * 128:
                                            b * 512 + jt * 256 + ia * 128 + 128],
                                    rhs=v_cur[:, jt * C + b: jt * C + b + 1],
                                    start=(jt == 0), stop=(jt == 1),
                                )
                    u_t = uvp.tile([128, 2 * C], f32, name="u_t")
                    nc.vector.tensor_scalar_max(u_t[:], ps_u[:], EPS)
                    u_cur = uvp.tile([128, 2 * C], f32, name="u_cur")
                    nc.vector.reciprocal(u_cur[:], u_t[:])

                    ps_v = psuv.tile([128, 2 * C], f32, name="ps_v")
                    for b in range(C):
                        for jt in range(2):
                            for ia in range(2):
                                nc.tensor.matmul(
                                    ps_v[:, jt * C + b: jt * C + b + 1],
                                    lhsT=e0[:, b * 512 + ia * 256 + jt * 128:
                                            b * 512 + ia * 256 + jt * 128 + 128],
                                    rhs=u_cur[:, ia * C + b: ia * C + b + 1],
                                    start=(ia == 0), stop=(ia == 1),
                                )
                    v_t = uvp.tile([128, 2 * C], f32, name="v_t")
                    nc.vector.tensor_scalar_max(v_t[:], ps_v[:], EPS)
                    v_cur = uvp.tile([128, 2 * C], f32, name="v_cur")
                    nc.vector.reciprocal(v_cur[:], v_t[:])

                # ---- materialize out = e0 * (u outer v) * OUT_SCALE ----
                # v columns -> rows (one PE transpose per half, whole cohort)
                ps_vr = psvr.tile([128, 256], f32, name="ps_vr")
                for jt in range(2):
                    nc.tensor.transpose(
                        ps_vr[0:C, jt * 128:(jt + 1) * 128],
                        v_cur[:, jt * C:(jt + 1) * C],
                        ident_sb[:],
                    )
                vrow = vrowp.tile([128, 256], f32, name="vrow")
                nc.vector.tensor_copy(vrow[0:C, :], ps_vr[0:C, :])

                for b in range(C):
                    ps_b = psbc.tile([128, 256], f32, name="ps_b")
                    # [128,256] per-sample broadcast of v-row, scaled by OUT_SCALE
                    nc.tensor.matmul(
                        ps_b[:], lhsT=sel_sb[0:C, b * 128:(b + 1) * 128],
                        rhs=vrow[0:C, :], start=True, stop=True,
                    )
                    for ia in range(2):
                        w = wp.tile([128, 256], f32, name="w")
                        nc.scalar.activation(
                            w[:], ps_b[:], AF.Copy,
                            scale=u_cur[:, ia * C + b: ia * C + b + 1],
                        )
                        sl = slice(b * 512 + ia * 256, b * 512 + (ia + 1) * 256)
                        nc.vector.tensor_mul(e0[:, sl], e0[:, sl], w[:])

                dst = out[s0:s0 + C].rearrange("b (h p) j -> p b h j", p=128)
                nc.sync.dma_start(dst, e0[:].rearrange("p (b h j) -> p b h j", h=2, j=256))

    nc.compile()
    return nc


def _get_nc():
    global _NC_CACHE
    if _NC_CACHE is None:
        _NC_CACHE = _build_nc()
    return _NC_CACHE


def _prep_in_maps(logits, free_agents_num, tasks_num):
    logits = np.asarray(logits, dtype=np.float32)
    free = np.asarray(free_agents_num).astype(np.int64)
    tasks = np.asarray(tasks_num).astype(np.int64)
    row_ok = np.arange(A, dtype=np.int64)[None, :] < free[:, None]   # [B, A]
    col_ok = np.arange(T, dtype=np.int64)[None, :] < tasks[:, None]  # [B, T]
    mask = row_ok[:, :, None] & col_ok[:, None, :]
    lgm = np.where(mask, logits, MASKVAL).astype(np.float32)
    lgmT = np.ascontiguousarray(lgm.transpose(0, 2, 1))
    ident = np.eye(128, dtype=np.float32)
    sel = np.zeros((C, C * 128), dtype=np.float32)
    for b in range(C):
        sel[b, b * 128:(b + 1) * 128] = OUT_SCALE
    return [
        {
            "lg": np.ascontiguousarray(lgm[c * BPC:(c + 1) * BPC]),
            "lgT": lgmT[c * BPC:(c + 1) * BPC],
            "ident": ident,
            "sel": sel,
        }
        for c in range(NCORES)
    ]


def _run(logits, free_agents_num, tasks_num, **spmd_kwargs):
    from concourse.bass_utils import run_bass_kernel_spmd

    in_maps = _prep_in_maps(logits, free_agents_num, tasks_num)
    res = run_bass_kernel_spmd(
        _get_nc(), in_maps, core_ids=list(range(NCORES)), **spmd_kwargs
    )
    out = np.concatenate([r["out"] for r in res.results], axis=0)
    return out, res


def kernel(logits, free_agents_num, tasks_num):
    out, _ = _run(logits, free_agents_num, tasks_num)
    return out
